# revision 10
# baseline (speedup 1.0000x reference)
"""Expert-choice MoE kernel for 8 Trainium2 NeuronCores (expert-parallel).

Decomposition (core e handles expert e):
  - router logits x . emb_e computed in fp32 on PE; top-8 token indices per
    batch row via DVE max8/max_index; token gather via indirect DMA.
  - expert MLP (two 3072x3072 GEMMs) in bf16 with fp32 PSUM accumulation.
  - sum_weights GEMM1 column-sharded (each core owns 1536 columns of sw_w1);
    the tiny (8,64) partial logits are AllReduced, softmaxed locally.
  - er * w[:, e] contributions AllReduced -> ws on every core.
  - classification head sharded: GEMM1 column-shard (384 cols of ch_w1),
    GEMM2 contraction-shard (384 rows of ch_w2); per-core (64,1000) partials
    are summed on the host (+ ch_b2).

Weights stream from HBM in natural [in, out] layout: one DMA per 128-row
k-chunk ([128, 3072] bf16 = 6KB contiguous per partition), consumed as the
moving matmul operand by 6 (or 3) live PSUM accumulators (k-outer loop).
"""

import numpy as np
import ml_dtypes

import concourse.bass as bass
from concourse import bacc
import concourse.mybir as mybir
import concourse.tile as tile
from concourse.bass import ts, ds
from concourse.bass_utils import run_bass_kernel_spmd
from concourse.masks import make_identity

B, N, D, E, K, C = 64, 32, 384, 8, 8, 1000
KD, ND = K * D, N * D          # 3072, 12288
P = 128
NTOK = B * N                   # 2048
SWC = ND // E                  # 1536 sum-weights columns per core
CH1C = KD // E                 # 384 head-GEMM1 columns per core
KCE = KD // P                  # 24 k-chunks, expert GEMMs
KCS = ND // P                  # 96 k-chunks, sum-weights GEMM1
KCH = SWC // P                 # 12 k-chunks, z GEMM
NCORES = 8

F32 = mybir.dt.float32
BF16 = mybir.dt.bfloat16
U32 = mybir.dt.uint32
GELU = mybir.ActivationFunctionType.Gelu
EXP = mybir.ActivationFunctionType.Exp
X_AX = mybir.AxisListType.X
ADD = mybir.AluOpType.add
bf16 = ml_dtypes.bfloat16


def _build(include_bias: bool) -> bass.Bass:
    nc = bacc.Bacc("TRN2", num_devices=NCORES)

    xt = nc.dram_tensor("xt", [P, 3, NTOK + 1], F32, kind="ExternalInput")
    x2b = nc.dram_tensor("x2b", [NTOK, D], BF16, kind="ExternalInput")
    xft = nc.dram_tensor("xft", [P, KCS, B], BF16, kind="ExternalInput")
    w1 = nc.dram_tensor("w1", [KD, KD], BF16, kind="ExternalInput")
    w2 = nc.dram_tensor("w2", [KD, KD], BF16, kind="ExternalInput")
    sw1 = nc.dram_tensor("sw1", [ND, SWC], BF16, kind="ExternalInput")
    sw2 = nc.dram_tensor("sw2", [P, KCH, E], BF16, kind="ExternalInput")
    ch1 = nc.dram_tensor("ch1", [KD, CH1C], BF16, kind="ExternalInput")
    ch2 = nc.dram_tensor("ch2", [3, P, C], BF16, kind="ExternalInput")
    oh = nc.dram_tensor("oh", [B, E], F32, kind="ExternalInput")
    if include_bias:
        b1d = nc.dram_tensor("b1d", [1, KD], F32, kind="ExternalInput")
        b2d = nc.dram_tensor("b2d", [1, KD], F32, kind="ExternalInput")
        swb1d = nc.dram_tensor("swb1d", [1, SWC], F32, kind="ExternalInput")
        swb2d = nc.dram_tensor("swb2d", [1, E], F32, kind="ExternalInput")
        chb1d = nc.dram_tensor("chb1d", [1, CH1C], F32, kind="ExternalInput")
    outp = nc.dram_tensor("outp", [B, C], F32, kind="ExternalOutput")

    with tile.TileContext(nc) as tc:
        with (
            tc.tile_pool(name="consts", bufs=1) as consts,
            tc.tile_pool(name="acts", bufs=1) as acts,
            tc.tile_pool(name="wpool", bufs=7) as wpool,
            tc.tile_pool(name="ps_mm", bufs=6, space="PSUM") as ps_mm,
            tc.tile_pool(name="ps_tr", bufs=2, space="PSUM") as ps_tr,
            tc.tile_pool(name="dram", bufs=1, space="DRAM") as dram,
        ):
            # ---- constants / persistent activations ----
            ident = consts.tile([P, P], BF16)
            make_identity(nc, ident[:])
            identf = consts.tile([P, P], F32)
            make_identity(nc, identf[:])
            xt_sb = acts.tile([P, 3, NTOK + 1], F32)
            nc.sync.dma_start(xt_sb[:], xt[:])
            xft_sb = consts.tile([P, KCS, B], BF16)
            nc.sync.dma_start(xft_sb[:], xft[:])
            sw2_sb = consts.tile([P, KCH, E], BF16)
            nc.sync.dma_start(sw2_sb[:], sw2[:])
            oh_sb = consts.tile([B, E], F32)
            nc.sync.dma_start(oh_sb[:], oh[:])
            ch2_sb = consts.tile([P, 3, C], BF16)
            nc.sync.dma_start(ch2_sb[:], ch2[:].rearrange("c p f -> p c f"))
            pwarm = ps_tr.tile([P, B], BF16, name="pwarm", tag="pt")
            nc.tensor.transpose(pwarm[:32, :32], ident[:32, :32], ident[:32, :32])
            if include_bias:
                b1_sb = consts.tile([B, KD], F32)
                nc.sync.dma_start(b1_sb[:], b1d[0:1, :].to_broadcast([B, KD]))
                b2_sb = consts.tile([B, KD], F32)
                nc.sync.dma_start(b2_sb[:], b2d[0:1, :].to_broadcast([B, KD]))
                swb1_sb = consts.tile([B, SWC], F32)
                nc.sync.dma_start(swb1_sb[:], swb1d[0:1, :].to_broadcast([B, SWC]))
                swb2_sb = consts.tile([B, E], F32)
                nc.sync.dma_start(swb2_sb[:], swb2d[0:1, :].to_broadcast([B, E]))
                chb1_sb = consts.tile([B, CH1C], F32)
                nc.sync.dma_start(chb1_sb[:], chb1d[0:1, :].to_broadcast([B, CH1C]))

            def sw_chunk(c, pms):
                wt = wpool.tile([P, KD], BF16, name="wt", tag="wt")
                nc.sync.dma_start(wt[:, :SWC], sw1[ts(c, P), :])
                for n in range(3):
                    nc.tensor.matmul(
                        pms[n][:], xft_sb[:, c, :], wt[:, ts(n, 512)],
                        start=(c == 0), stop=(c == KCS - 1),
                    )

            # ---- sum-weights GEMM1 (first chunks warm the PE before router) ----
            pms = [ps_mm.tile([B, 512], F32, name=f"pms{n}", tag="pm")
                   for n in range(3)]
            for c in range(8):
                sw_chunk(c, pms)

            # ---- router: logits = x @ emb_e, fp32 (emb packed as col 2048) ----
            lg_flat = acts.tile([1, NTOK], F32)
            for nt in range(4):
                pr = ps_mm.tile([B, 512], F32, name="pr", tag="pm")
                for c in range(3):
                    nc.tensor.matmul(
                        pr[:1, :], xt_sb[:, c, NTOK : NTOK + 1],
                        xt_sb[:, c, ts(nt, 512)],
                        start=(c == 0), stop=(c == 2),
                    )
                nc.vector.tensor_copy(lg_flat[:, ts(nt, 512)], pr[:1, :])
            lg_dram = dram.tile([1, NTOK], F32)
            nc.scalar.dma_start(lg_dram[:], lg_flat[:])
            lg_bn = acts.tile([B, N], F32)
            nc.scalar.dma_start(lg_bn[:], lg_dram[:].rearrange("x (b n) -> (x b) n", b=B))

            # ---- top-8 tokens per row + gather (overlaps sw streaming) ----
            vals8 = acts.tile([B, 8], F32)
            idx8 = acts.tile([B, 8], U32)
            nc.vector.max(out=vals8[:], in_=lg_bn[:])
            nc.vector.max_index(out=idx8[:], in_max=vals8[:], in_values=lg_bn[:])
            base = acts.tile([B, 1], U32)
            nc.gpsimd.iota(base[:], pattern=[[0, 1]], base=0, channel_multiplier=N)
            off = acts.tile([B, 8], U32)
            nc.vector.tensor_tensor(
                out=off[:], in0=idx8[:], in1=base[:].to_broadcast([B, 8]), op=ADD
            )
            sel = acts.tile([B, K, D], BF16)
            for k in range(K):
                nc.gpsimd.indirect_dma_start(
                    out=sel[:, k, :], out_offset=None,
                    in_=x2b[:],
                    in_offset=bass.IndirectOffsetOnAxis(ap=off[:, k : k + 1], axis=0),
                )
            sel_flat = sel[:].rearrange("b k d -> b (k d)")

            # ---- rest of the sum-weights stream ----
            for c in range(8, KCS):
                sw_chunk(c, pms)
            h1 = acts.tile([B, SWC], BF16)
            for n in range(3):
                if include_bias:
                    nc.vector.tensor_add(pms[n][:], pms[n][:], swb1_sb[:, ts(n, 512)])
                nc.scalar.activation(h1[:, ts(n, 512)], pms[n][:], GELU)
            h1T = acts.tile([P, KCH, B], BF16)
            for c in range(KCH):
                pt = ps_tr.tile([P, B], BF16, name="pt", tag="pt")
                nc.tensor.transpose(pt[:], h1[:, ts(c, P)], ident[:B, :B])
                nc.vector.tensor_copy(h1T[:, c, :], pt[:])
            pz = ps_mm.tile([B, 512], F32, name="pz", tag="pm")
            for c in range(KCH):
                nc.tensor.matmul(
                    pz[:E, :B], sw2_sb[:, c, :], h1T[:, c, :],
                    start=(c == 0), stop=(c == KCH - 1),
                )
            zT_sb = acts.tile([E, B], F32)
            nc.vector.tensor_copy(zT_sb[:], pz[:E, :B])
            zin = dram.tile([E, B], F32)
            zout = dram.tile([E, B], F32)
            nc.gpsimd.dma_start(zin[:], zT_sb[:])
            nc.gpsimd.collective_compute(
                "AllReduce", ADD, replica_groups=[list(range(NCORES))],
                ins=[zin[:].opt()], outs=[zout[:].opt()],
            )

            # selT chunks [128, 24, 64] for expert GEMM1 lhsT
            selT = acts.tile([P, KCE, B], BF16)
            for c in range(KCE):
                pt = ps_tr.tile([P, B], BF16, name="pt", tag="pt")
                nc.tensor.transpose(pt[:], sel_flat[:, ts(c, P)], ident[:B, :B])
                nc.vector.tensor_copy(selT[:, c, :], pt[:])

            # ---- expert GEMM1: h = gelu(selT.T @ w1_e) ----
            h = acts.tile([B, KD], BF16)
            pme = [ps_mm.tile([B, 512], F32, name=f"pme{n}", tag="pm")
                   for n in range(6)]
            for c in range(KCE):
                wt = wpool.tile([P, KD], BF16, name="wt", tag="wt")
                nc.sync.dma_start(wt[:], w1[ts(c, P), :])
                for n in range(6):
                    nc.tensor.matmul(
                        pme[n][:], selT[:, c, :], wt[:, ts(n, 512)],
                        start=(c == 0), stop=(c == KCE - 1),
                    )
            last_gelu = None
            for n in range(6):
                if include_bias:
                    nc.vector.tensor_add(pme[n][:], pme[n][:], b1_sb[:, ts(n, 512)])
                last_gelu = nc.scalar.activation(h[:, ts(n, 512)], pme[n][:], GELU)
            hT = acts.tile([P, KCE, B], BF16)
            last_htc = None
            for c in range(KCE):
                pt = ps_tr.tile([P, B], BF16, name="pt", tag="pt")
                nc.tensor.transpose(pt[:], h[:, ts(c, P)], ident[:B, :B])
                last_htc = nc.vector.tensor_copy(hT[:, c, :], pt[:])

            # softmax over experts, then w_e = sum(w * onehot_e). Ordering
            # deps keep the z-AllReduce consumers BEHIND the expert-GEMM work
            # on the ACT/DVE queues (the scheduler would otherwise hoist them
            # and park those queues on the collective).
            zb = acts.tile([B, E], F32)
            nc.gpsimd.dma_start(zb[:], zout[:].rearrange("e b -> b e"))
            if include_bias:
                nc.vector.tensor_add(zb[:], zb[:], swb2_sb[:])
            mx = acts.tile([B, 1], F32)
            mx_i = nc.vector.reduce_max(mx[:], zb[:], axis=X_AX)
            tile.add_dep_helper(mx_i.ins, last_htc.ins, sync=False,
                                reason="softmax after hT copies on DVE")
            nmx = acts.tile([B, 1], F32)
            nc.vector.tensor_scalar_mul(nmx[:], mx[:], -1.0)
            exps = acts.tile([B, E], F32)
            exp_i = nc.scalar.activation(exps[:], zb[:], EXP, bias=nmx[:])
            tile.add_dep_helper(exp_i.ins, last_gelu.ins, sync=False,
                                reason="Exp after expert gelus on ACT")
            sm = acts.tile([B, 1], F32)
            nc.vector.reduce_sum(sm[:], exps[:], axis=X_AX)
            rs = acts.tile([B, 1], F32)
            nc.vector.reciprocal(rs[:], sm[:])
            wv = acts.tile([B, E], F32)
            nc.vector.tensor_scalar_mul(wv[:], exps[:], rs[:])
            t8 = acts.tile([B, E], F32)
            nc.vector.tensor_mul(out=t8[:], in0=wv[:], in1=oh_sb[:])
            we = acts.tile([B, 1], F32)
            nc.vector.reduce_sum(we[:], t8[:], axis=X_AX)

            # ---- expert GEMM2 in 3 column chunks, each with its own AllReduce ----
            NCHUNK, CW = 3, KD // 3          # 3 chunks x 1024 columns
            er = acts.tile([B, KD], F32)
            cins = [dram.tile([B, CW], F32, name=f"cin{i}") for i in range(NCHUNK)]
            wsouts = [dram.tile([B, CW], F32, name=f"wsout{i}")
                      for i in range(NCHUNK)]
            for j in range(NCHUNK):
                pme2 = [ps_mm.tile([B, 512], F32, name=f"pme2{j}{n}", tag="pm")
                        for n in range(2)]
                for c in range(KCE):
                    wt = wpool.tile([P, KD], BF16, name="wt", tag="wt")
                    nc.sync.dma_start(
                        wt[:, :CW], w2[ts(c, P), ds(j * CW, CW)])
                    for n in range(2):
                        nc.tensor.matmul(
                            pme2[n][:], hT[:, c, :], wt[:, ts(n, 512)],
                            start=(c == 0), stop=(c == KCE - 1),
                        )
                for n in range(2):
                    col = j * CW + n * 512
                    if include_bias:
                        nc.vector.tensor_add(
                            pme2[n][:], pme2[n][:], b2_sb[:, ds(col, 512)])
                    # weighted contribution w[:, e] * er folded into the copy-out
                    nc.vector.tensor_scalar_mul(
                        er[:, ds(col, 512)], pme2[n][:], we[:])
                nc.scalar.dma_start(cins[j][:], er[:, ds(j * CW, CW)])
                nc.gpsimd.collective_compute(
                    "AllReduce", ADD, replica_groups=[list(range(NCORES))],
                    ins=[cins[j][:].opt()], outs=[wsouts[j][:].opt()],
                )

            # ---- prefetch ch1 into SBUF while the AllReduces fly ----
            ch1_sb = acts.tile([P, KCE, CH1C], BF16)
            for c in range(KCE):
                nc.sync.dma_start(ch1_sb[:, c, :], ch1[ts(c, P), :])
            ws = acts.tile([B, KD], F32)
            for j in range(NCHUNK):
                nc.scalar.dma_start(ws[:, ds(j * CW, CW)], wsouts[j][:])

            # ---- head GEMM1 (column shard): hh = gelu(ws @ ch1_e) ----
            wsT = acts.tile([P, KCE, B], BF16)
            hh = acts.tile([B, CH1C], BF16)
            pmh = ps_mm.tile([B, 512], F32, name="pmh", tag="pm")
            for j in range(NCHUNK):
                for cc in range(KCE // NCHUNK):
                    c = j * (KCE // NCHUNK) + cc
                    pt = ps_tr.tile([P, B], F32, name="ptf", tag="pt")
                    nc.tensor.transpose(pt[:], ws[:, ts(c, P)], identf[:B, :B])
                    nc.vector.tensor_copy(wsT[:, c, :], pt[:])
                    nc.tensor.matmul(
                        pmh[:, :CH1C], wsT[:, c, :], ch1_sb[:, c, :],
                        start=(c == 0), stop=(c == KCE - 1),
                    )
            if include_bias:
                nc.vector.tensor_add(pmh[:, :CH1C], pmh[:, :CH1C], chb1_sb[:])
            nc.scalar.activation(hh[:], pmh[:, :CH1C], GELU)

            # ---- head GEMM2 (contraction shard): out_part = hh @ ch2_e ----
            hhT = acts.tile([P, 3, B], BF16)
            for c in range(3):
                pt = ps_tr.tile([P, B], BF16, name="pt", tag="pt")
                nc.tensor.transpose(pt[:], hh[:, ts(c, P)], ident[:B, :B])
                nc.vector.tensor_copy(hhT[:, c, :], pt[:])
            outsb = acts.tile([B, C], F32)
            for nn in range(2):
                pmo = ps_mm.tile([B, 512], F32, name="pmo", tag="pm")
                for c in range(3):
                    nc.tensor.matmul(
                        pmo[:, :500], hhT[:, c, :], ch2_sb[:, c, ds(nn * 500, 500)],
                        start=(c == 0), stop=(c == 2),
                    )
                nc.vector.tensor_copy(outsb[:, ds(nn * 500, 500)], pmo[:, :500])
            nc.sync.dma_start(outp[:], outsb[:])

    nc.finalize()
    return nc


_NC_CACHE: dict = {}


def _get_nc(include_bias: bool) -> bass.Bass:
    if include_bias not in _NC_CACHE:
        _NC_CACHE[include_bias] = _build(include_bias)
    return _NC_CACHE[include_bias]


def _pack_inputs(inputs: dict, include_bias: bool) -> list[dict]:
    f32 = np.float32
    x = np.ascontiguousarray(np.asarray(inputs["x"], dtype=f32))      # (64,32,384)
    expert_emb = np.asarray(inputs["expert_emb"], dtype=f32)          # (8,384)
    w1 = np.asarray(inputs["w1"])                                     # (8,3072,3072)
    w2 = np.asarray(inputs["w2"])
    sw_w1 = np.asarray(inputs["sw_w1"])                               # (12288,12288)
    sw_w2 = np.asarray(inputs["sw_w2"])                               # (12288,8)
    ch_w1 = np.asarray(inputs["ch_w1"])                               # (3072,3072)
    ch_w2 = np.asarray(inputs["ch_w2"])                               # (3072,1000)

    x2 = x.reshape(NTOK, D)
    xt_base = x2.T.reshape(3, P, NTOK).transpose(1, 0, 2)             # (128,3,2048)
    x2b = x2.astype(bf16)                                             # (2048,384)
    xf = x.reshape(B, ND)
    xft_p = np.ascontiguousarray(
        xf.T.reshape(KCS, P, B).transpose(1, 0, 2)).astype(bf16)      # (128,96,64)

    ch1_full = ch_w1.reshape(KD, E, CH1C)                             # col shards
    ch2_full = ch_w2.reshape(E, CH1C, C)                              # row shards

    in_maps = []
    for e in range(NCORES):
        emb_p = expert_emb[e].reshape(3, P).T                          # (128,3)
        xt_p = np.ascontiguousarray(
            np.concatenate([xt_base, emb_p[:, :, None]], axis=2), dtype=f32)
        sw1_e = np.ascontiguousarray(
            sw_w1[:, e * SWC:(e + 1) * SWC]).astype(bf16)              # (12288,1536)
        sw2_e = np.ascontiguousarray(sw_w2[e * SWC:(e + 1) * SWC, :])  # (1536,8)
        sw2_p = np.ascontiguousarray(
            sw2_e.reshape(KCH, P, E).transpose(1, 0, 2)).astype(bf16)  # (128,12,8)
        ch1_p = np.ascontiguousarray(ch1_full[:, e, :]).astype(bf16)   # (3072,384)
        ch2_p = np.ascontiguousarray(
            ch2_full[e].reshape(3, P, C)).astype(bf16)                 # (3,128,1000)
        oh_p = np.zeros((B, E), dtype=f32)
        oh_p[:, e] = 1.0
        m = {
            "xt": xt_p, "x2b": x2b, "xft": xft_p,
            "w1": w1[e].astype(bf16), "w2": w2[e].astype(bf16),
            "sw1": sw1_e, "sw2": sw2_p, "ch1": ch1_p, "ch2": ch2_p, "oh": oh_p,
        }
        if include_bias:
            m["b1d"] = np.asarray(inputs["b1"][e], f32).reshape(1, KD)
            m["b2d"] = np.asarray(inputs["b2"][e], f32).reshape(1, KD)
            m["swb1d"] = np.asarray(
                inputs["sw_b1"], f32).reshape(1, ND)[:, e * SWC:(e + 1) * SWC]
            m["swb2d"] = np.asarray(inputs["sw_b2"], f32).reshape(1, E)
            m["chb1d"] = np.asarray(
                inputs["ch_b1"], f32).reshape(1, KD)[:, e * CH1C:(e + 1) * CH1C]
        in_maps.append(m)
    return in_maps


def _need_bias(inputs) -> bool:
    return any(
        float(np.abs(np.asarray(inputs[k])).max()) != 0.0
        for k in ("b1", "b2", "sw_b1", "sw_b2", "ch_b1")
    )


def run(inputs: dict, **run_kwargs):
    """Run on the 8 cores; returns (full_output, BassKernelResults)."""
    include_bias = _need_bias(inputs)
    nc = _get_nc(include_bias)
    in_maps = _pack_inputs(inputs, include_bias)
    res = run_bass_kernel_spmd(nc, in_maps, core_ids=list(range(NCORES)), **run_kwargs)
    out = np.zeros((B, C), dtype=np.float64)
    for e in range(NCORES):
        out += res.results[e]["outp"].astype(np.float64)
    out += np.asarray(inputs["ch_b2"], np.float64)
    return out.astype(np.float32), res


def kernel(**inputs) -> np.ndarray:
    out, _ = run(inputs)
    return out


# revision 11
# speedup vs baseline: 1.0368x; 1.0368x over previous
"""Expert-choice MoE kernel for 8 Trainium2 NeuronCores (expert-parallel).

Decomposition (core e handles expert e):
  - router logits x . emb_e computed in fp32 on PE; top-8 token indices per
    batch row via DVE max8/max_index; token gather via indirect DMA.
  - expert MLP (two 3072x3072 GEMMs) in bf16 with fp32 PSUM accumulation.
  - sum_weights GEMM1 column-sharded (each core owns 1536 columns of sw_w1);
    the tiny (8,64) partial logits are AllReduced, softmaxed locally.
  - er * w[:, e] contributions AllReduced -> ws on every core.
  - classification head sharded: GEMM1 column-shard (384 cols of ch_w1),
    GEMM2 contraction-shard (384 rows of ch_w2); per-core (64,1000) partials
    are summed on the host (+ ch_b2).

Weights stream from HBM in natural [in, out] layout: one DMA per 128-row
k-chunk ([128, 3072] bf16 = 6KB contiguous per partition), consumed as the
moving matmul operand by 6 (or 3) live PSUM accumulators (k-outer loop).
"""

import numpy as np
import ml_dtypes

import concourse.bass as bass
from concourse import bacc
import concourse.mybir as mybir
import concourse.tile as tile
from concourse.bass import ts, ds
from concourse.bass_utils import run_bass_kernel_spmd
from concourse.masks import make_identity

B, N, D, E, K, C = 64, 32, 384, 8, 8, 1000
KD, ND = K * D, N * D          # 3072, 12288
P = 128
NTOK = B * N                   # 2048
SWC = ND // E                  # 1536 sum-weights columns per core
CH1C = KD // E                 # 384 head-GEMM1 columns per core
KCE = KD // P                  # 24 k-chunks, expert GEMMs
KCS = ND // P                  # 96 k-chunks, sum-weights GEMM1
KCH = SWC // P                 # 12 k-chunks, z GEMM
NCORES = 8

F32 = mybir.dt.float32
BF16 = mybir.dt.bfloat16
U32 = mybir.dt.uint32
GELU = mybir.ActivationFunctionType.Gelu
EXP = mybir.ActivationFunctionType.Exp
X_AX = mybir.AxisListType.X
ADD = mybir.AluOpType.add
bf16 = ml_dtypes.bfloat16


def _build(include_bias: bool) -> bass.Bass:
    nc = bacc.Bacc("TRN2", num_devices=NCORES)

    xt = nc.dram_tensor("xt", [P, 3, NTOK + 1], F32, kind="ExternalInput")
    x2b = nc.dram_tensor("x2b", [NTOK, D], BF16, kind="ExternalInput")
    xft = nc.dram_tensor("xft", [P, KCS, B], BF16, kind="ExternalInput")
    w1 = nc.dram_tensor("w1", [KD, KD], BF16, kind="ExternalInput")
    w2 = nc.dram_tensor("w2", [KD, KD], BF16, kind="ExternalInput")
    sw1 = nc.dram_tensor("sw1", [ND, SWC], BF16, kind="ExternalInput")
    sw2 = nc.dram_tensor("sw2", [P, KCH, E], BF16, kind="ExternalInput")
    ch1 = nc.dram_tensor("ch1", [KD, CH1C], BF16, kind="ExternalInput")
    ch2 = nc.dram_tensor("ch2", [3, P, C], BF16, kind="ExternalInput")
    oh = nc.dram_tensor("oh", [B, E], F32, kind="ExternalInput")
    if include_bias:
        b1d = nc.dram_tensor("b1d", [1, KD], F32, kind="ExternalInput")
        b2d = nc.dram_tensor("b2d", [1, KD], F32, kind="ExternalInput")
        swb1d = nc.dram_tensor("swb1d", [1, SWC], F32, kind="ExternalInput")
        swb2d = nc.dram_tensor("swb2d", [1, E], F32, kind="ExternalInput")
        chb1d = nc.dram_tensor("chb1d", [1, CH1C], F32, kind="ExternalInput")
    outp = nc.dram_tensor("outp", [B, C], F32, kind="ExternalOutput")

    with tile.TileContext(nc) as tc:
        with (
            tc.tile_pool(name="consts", bufs=1) as consts,
            tc.tile_pool(name="acts", bufs=1) as acts,
            tc.tile_pool(name="wpool", bufs=7) as wpool,
            tc.tile_pool(name="ps_mm", bufs=6, space="PSUM") as ps_mm,
            tc.tile_pool(name="ps_tr", bufs=2, space="PSUM") as ps_tr,
            tc.tile_pool(name="dram", bufs=1, space="DRAM") as dram,
        ):
            # ---- constants / persistent activations ----
            ident = consts.tile([P, P], BF16)
            make_identity(nc, ident[:])
            identf = consts.tile([P, P], F32)
            make_identity(nc, identf[:])
            xt_sb = acts.tile([P, 3, NTOK + 1], F32)
            nc.sync.dma_start(xt_sb[:], xt[:])
            xft_sb = consts.tile([P, KCS, B], BF16)
            nc.sync.dma_start(xft_sb[:], xft[:])
            pwarm = ps_tr.tile([P, B], BF16, name="pwarm", tag="pt")
            nc.tensor.transpose(pwarm[:32, :32], ident[:32, :32], ident[:32, :32])
            if include_bias:
                b1_sb = consts.tile([B, KD], F32)
                nc.sync.dma_start(b1_sb[:], b1d[0:1, :].to_broadcast([B, KD]))
                b2_sb = consts.tile([B, KD], F32)
                nc.sync.dma_start(b2_sb[:], b2d[0:1, :].to_broadcast([B, KD]))
                swb1_sb = consts.tile([B, SWC], F32)
                nc.sync.dma_start(swb1_sb[:], swb1d[0:1, :].to_broadcast([B, SWC]))
                swb2_sb = consts.tile([B, E], F32)
                nc.sync.dma_start(swb2_sb[:], swb2d[0:1, :].to_broadcast([B, E]))
                chb1_sb = consts.tile([B, CH1C], F32)
                nc.sync.dma_start(chb1_sb[:], chb1d[0:1, :].to_broadcast([B, CH1C]))

            def sw_chunk(c, pms):
                wt = wpool.tile([P, KD], BF16, name="wt", tag="wt")
                nc.sync.dma_start(wt[:, :SWC], sw1[ts(c, P), :])
                for n in range(3):
                    nc.tensor.matmul(
                        pms[n][:], xft_sb[:, c, :], wt[:, ts(n, 512)],
                        start=(c == 0), stop=(c == KCS - 1),
                    )

            # ---- sum-weights GEMM1 (first chunks warm the PE before router) ----
            pms = [ps_mm.tile([B, 512], F32, name=f"pms{n}", tag="pm")
                   for n in range(3)]
            for c in range(8):
                sw_chunk(c, pms)

            # ---- router: logits = x @ emb_e, fp32 (emb packed as col 2048) ----
            lg_flat = acts.tile([1, NTOK], F32)
            for nt in range(4):
                pr = ps_mm.tile([B, 512], F32, name="pr", tag="pm")
                for c in range(3):
                    nc.tensor.matmul(
                        pr[:1, :], xt_sb[:, c, NTOK : NTOK + 1],
                        xt_sb[:, c, ts(nt, 512)],
                        start=(c == 0), stop=(c == 2),
                    )
                nc.vector.tensor_copy(lg_flat[:, ts(nt, 512)], pr[:1, :])
            lg_dram = dram.tile([1, NTOK], F32)
            nc.scalar.dma_start(lg_dram[:], lg_flat[:])
            lg_bn = acts.tile([B, N], F32)
            nc.scalar.dma_start(lg_bn[:], lg_dram[:].rearrange("x (b n) -> (x b) n", b=B))

            # ---- top-8 tokens per row + gather (overlaps sw streaming) ----
            vals8 = acts.tile([B, 8], F32)
            idx8 = acts.tile([B, 8], U32)
            nc.vector.max(out=vals8[:], in_=lg_bn[:])
            nc.vector.max_index(out=idx8[:], in_max=vals8[:], in_values=lg_bn[:])
            base = acts.tile([B, 1], U32)
            nc.gpsimd.iota(base[:], pattern=[[0, 1]], base=0, channel_multiplier=N)
            off = acts.tile([B, 8], U32)
            nc.vector.tensor_tensor(
                out=off[:], in0=idx8[:], in1=base[:].to_broadcast([B, 8]), op=ADD
            )
            sel = acts.tile([B, K, D], BF16)
            for k in range(K):
                nc.gpsimd.indirect_dma_start(
                    out=sel[:, k, :], out_offset=None,
                    in_=x2b[:],
                    in_offset=bass.IndirectOffsetOnAxis(ap=off[:, k : k + 1], axis=0),
                )
            sel_flat = sel[:].rearrange("b k d -> b (k d)")

            # ---- rest of the sum-weights stream ----
            sw2_sb = consts.tile([P, KCH, E], BF16)
            nc.sync.dma_start(sw2_sb[:], sw2[:])
            oh_sb = consts.tile([B, E], F32)
            nc.sync.dma_start(oh_sb[:], oh[:])
            ch2_sb = consts.tile([P, 3, C], BF16)
            nc.sync.dma_start(ch2_sb[:], ch2[:].rearrange("c p f -> p c f"))
            for c in range(8, KCS):
                sw_chunk(c, pms)
            h1 = acts.tile([B, SWC], BF16)
            for n in range(3):
                if include_bias:
                    nc.vector.tensor_add(pms[n][:], pms[n][:], swb1_sb[:, ts(n, 512)])
                nc.scalar.activation(h1[:, ts(n, 512)], pms[n][:], GELU)
            h1T = acts.tile([P, KCH, B], BF16)
            for c in range(KCH):
                pt = ps_tr.tile([P, B], BF16, name="pt", tag="pt")
                nc.tensor.transpose(pt[:], h1[:, ts(c, P)], ident[:B, :B])
                nc.vector.tensor_copy(h1T[:, c, :], pt[:])
            pz = ps_mm.tile([B, 512], F32, name="pz", tag="pm")
            for c in range(KCH):
                nc.tensor.matmul(
                    pz[:E, :B], sw2_sb[:, c, :], h1T[:, c, :],
                    start=(c == 0), stop=(c == KCH - 1),
                )
            zT_sb = acts.tile([E, B], F32)
            nc.vector.tensor_copy(zT_sb[:], pz[:E, :B])
            zin = dram.tile([E, B], F32)
            zout = dram.tile([E, B], F32)
            nc.gpsimd.dma_start(zin[:], zT_sb[:])
            nc.gpsimd.collective_compute(
                "AllReduce", ADD, replica_groups=[list(range(NCORES))],
                ins=[zin[:].opt()], outs=[zout[:].opt()],
            )

            # selT chunks [128, 24, 64] for expert GEMM1 lhsT
            selT = acts.tile([P, KCE, B], BF16)
            for c in range(KCE):
                pt = ps_tr.tile([P, B], BF16, name="pt", tag="pt")
                nc.tensor.transpose(pt[:], sel_flat[:, ts(c, P)], ident[:B, :B])
                nc.vector.tensor_copy(selT[:, c, :], pt[:])

            # ---- expert GEMM1: h = gelu(selT.T @ w1_e) ----
            h = acts.tile([B, KD], BF16)
            pme = [ps_mm.tile([B, 512], F32, name=f"pme{n}", tag="pm")
                   for n in range(6)]
            for c in range(KCE):
                wt = wpool.tile([P, KD], BF16, name="wt", tag="wt")
                nc.sync.dma_start(wt[:], w1[ts(c, P), :])
                for n in range(6):
                    nc.tensor.matmul(
                        pme[n][:], selT[:, c, :], wt[:, ts(n, 512)],
                        start=(c == 0), stop=(c == KCE - 1),
                    )
            last_gelu = None
            for n in range(6):
                if include_bias:
                    nc.vector.tensor_add(pme[n][:], pme[n][:], b1_sb[:, ts(n, 512)])
                last_gelu = nc.scalar.activation(h[:, ts(n, 512)], pme[n][:], GELU)
            hT = acts.tile([P, KCE, B], BF16)
            last_htc = None
            for c in range(KCE):
                pt = ps_tr.tile([P, B], BF16, name="pt", tag="pt")
                nc.tensor.transpose(pt[:], h[:, ts(c, P)], ident[:B, :B])
                last_htc = nc.vector.tensor_copy(hT[:, c, :], pt[:])

            # softmax over experts, then w_e = sum(w * onehot_e). Ordering
            # deps keep the z-AllReduce consumers BEHIND the expert-GEMM work
            # on the ACT/DVE queues (the scheduler would otherwise hoist them
            # and park those queues on the collective).
            zb = acts.tile([B, E], F32)
            nc.gpsimd.dma_start(zb[:], zout[:].rearrange("e b -> b e"))
            if include_bias:
                nc.vector.tensor_add(zb[:], zb[:], swb2_sb[:])
            mx = acts.tile([B, 1], F32)
            mx_i = nc.vector.reduce_max(mx[:], zb[:], axis=X_AX)
            tile.add_dep_helper(mx_i.ins, last_htc.ins, sync=False,
                                reason="softmax after hT copies on DVE")
            nmx = acts.tile([B, 1], F32)
            nc.vector.tensor_scalar_mul(nmx[:], mx[:], -1.0)
            exps = acts.tile([B, E], F32)
            exp_i = nc.scalar.activation(exps[:], zb[:], EXP, bias=nmx[:])
            tile.add_dep_helper(exp_i.ins, last_gelu.ins, sync=False,
                                reason="Exp after expert gelus on ACT")
            sm = acts.tile([B, 1], F32)
            nc.vector.reduce_sum(sm[:], exps[:], axis=X_AX)
            rs = acts.tile([B, 1], F32)
            nc.vector.reciprocal(rs[:], sm[:])
            wv = acts.tile([B, E], F32)
            nc.vector.tensor_scalar_mul(wv[:], exps[:], rs[:])
            t8 = acts.tile([B, E], F32)
            nc.vector.tensor_mul(out=t8[:], in0=wv[:], in1=oh_sb[:])
            we = acts.tile([B, 1], F32)
            nc.vector.reduce_sum(we[:], t8[:], axis=X_AX)

            # ---- expert GEMM2 in 3 column chunks, each with its own AllReduce ----
            NCHUNK, CW = 2, KD // 2          # 2 chunks x 1536 columns
            er = acts.tile([B, KD], F32)
            cins = [dram.tile([B, CW], F32, name=f"cin{i}") for i in range(NCHUNK)]
            wsouts = [dram.tile([B, CW], F32, name=f"wsout{i}")
                      for i in range(NCHUNK)]
            for j in range(NCHUNK):
                pme2 = [ps_mm.tile([B, 512], F32, name=f"pme2{j}{n}", tag="pm")
                        for n in range(3)]
                for c in range(KCE):
                    wt = wpool.tile([P, KD], BF16, name="wt", tag="wt")
                    nc.sync.dma_start(
                        wt[:, :CW], w2[ts(c, P), ds(j * CW, CW)])
                    for n in range(3):
                        nc.tensor.matmul(
                            pme2[n][:], hT[:, c, :], wt[:, ts(n, 512)],
                            start=(c == 0), stop=(c == KCE - 1),
                        )
                for n in range(3):
                    col = j * CW + n * 512
                    if include_bias:
                        nc.vector.tensor_add(
                            pme2[n][:], pme2[n][:], b2_sb[:, ds(col, 512)])
                    # weighted contribution w[:, e] * er folded into the copy-out
                    nc.vector.tensor_scalar_mul(
                        er[:, ds(col, 512)], pme2[n][:], we[:])
                nc.scalar.dma_start(cins[j][:], er[:, ds(j * CW, CW)])
                nc.gpsimd.collective_compute(
                    "AllReduce", ADD, replica_groups=[list(range(NCORES))],
                    ins=[cins[j][:].opt()], outs=[wsouts[j][:].opt()],
                )

            # ---- prefetch ch1 into SBUF while the AllReduces fly ----
            ch1_sb = acts.tile([P, KCE, CH1C], BF16)
            for c in range(KCE):
                nc.sync.dma_start(ch1_sb[:, c, :], ch1[ts(c, P), :])
            ws = acts.tile([B, KD], F32)
            for j in range(NCHUNK):
                nc.scalar.dma_start(ws[:, ds(j * CW, CW)], wsouts[j][:])

            # ---- head GEMM1 (column shard): hh = gelu(ws @ ch1_e) ----
            wsT = acts.tile([P, KCE, B], BF16)
            hh = acts.tile([B, CH1C], BF16)
            pmh = ps_mm.tile([B, 512], F32, name="pmh", tag="pm")
            for j in range(NCHUNK):
                for cc in range(KCE // NCHUNK):
                    c = j * (KCE // NCHUNK) + cc
                    pt = ps_tr.tile([P, B], F32, name="ptf", tag="pt")
                    nc.tensor.transpose(pt[:], ws[:, ts(c, P)], identf[:B, :B])
                    nc.vector.tensor_copy(wsT[:, c, :], pt[:])
                    nc.tensor.matmul(
                        pmh[:, :CH1C], wsT[:, c, :], ch1_sb[:, c, :],
                        start=(c == 0), stop=(c == KCE - 1),
                    )
            if include_bias:
                nc.vector.tensor_add(pmh[:, :CH1C], pmh[:, :CH1C], chb1_sb[:])
            nc.scalar.activation(hh[:], pmh[:, :CH1C], GELU)

            # ---- head GEMM2 (contraction shard): out_part = hh @ ch2_e ----
            hhT = acts.tile([P, 3, B], BF16)
            for c in range(3):
                pt = ps_tr.tile([P, B], BF16, name="pt", tag="pt")
                nc.tensor.transpose(pt[:], hh[:, ts(c, P)], ident[:B, :B])
                nc.vector.tensor_copy(hhT[:, c, :], pt[:])
            outsb = acts.tile([B, C], F32)
            for nn in range(2):
                pmo = ps_mm.tile([B, 512], F32, name="pmo", tag="pm")
                for c in range(3):
                    nc.tensor.matmul(
                        pmo[:, :500], hhT[:, c, :], ch2_sb[:, c, ds(nn * 500, 500)],
                        start=(c == 0), stop=(c == 2),
                    )
                nc.vector.tensor_copy(outsb[:, ds(nn * 500, 500)], pmo[:, :500])
            nc.sync.dma_start(outp[:], outsb[:])

    nc.finalize()
    return nc


_NC_CACHE: dict = {}


def _get_nc(include_bias: bool) -> bass.Bass:
    if include_bias not in _NC_CACHE:
        _NC_CACHE[include_bias] = _build(include_bias)
    return _NC_CACHE[include_bias]


def _pack_inputs(inputs: dict, include_bias: bool) -> list[dict]:
    f32 = np.float32
    x = np.ascontiguousarray(np.asarray(inputs["x"], dtype=f32))      # (64,32,384)
    expert_emb = np.asarray(inputs["expert_emb"], dtype=f32)          # (8,384)
    w1 = np.asarray(inputs["w1"])                                     # (8,3072,3072)
    w2 = np.asarray(inputs["w2"])
    sw_w1 = np.asarray(inputs["sw_w1"])                               # (12288,12288)
    sw_w2 = np.asarray(inputs["sw_w2"])                               # (12288,8)
    ch_w1 = np.asarray(inputs["ch_w1"])                               # (3072,3072)
    ch_w2 = np.asarray(inputs["ch_w2"])                               # (3072,1000)

    x2 = x.reshape(NTOK, D)
    xt_base = x2.T.reshape(3, P, NTOK).transpose(1, 0, 2)             # (128,3,2048)
    x2b = x2.astype(bf16)                                             # (2048,384)
    xf = x.reshape(B, ND)
    xft_p = np.ascontiguousarray(
        xf.T.reshape(KCS, P, B).transpose(1, 0, 2)).astype(bf16)      # (128,96,64)

    ch1_full = ch_w1.reshape(KD, E, CH1C)                             # col shards
    ch2_full = ch_w2.reshape(E, CH1C, C)                              # row shards

    in_maps = []
    for e in range(NCORES):
        emb_p = expert_emb[e].reshape(3, P).T                          # (128,3)
        xt_p = np.ascontiguousarray(
            np.concatenate([xt_base, emb_p[:, :, None]], axis=2), dtype=f32)
        sw1_e = np.ascontiguousarray(
            sw_w1[:, e * SWC:(e + 1) * SWC]).astype(bf16)              # (12288,1536)
        sw2_e = np.ascontiguousarray(sw_w2[e * SWC:(e + 1) * SWC, :])  # (1536,8)
        sw2_p = np.ascontiguousarray(
            sw2_e.reshape(KCH, P, E).transpose(1, 0, 2)).astype(bf16)  # (128,12,8)
        ch1_p = np.ascontiguousarray(ch1_full[:, e, :]).astype(bf16)   # (3072,384)
        ch2_p = np.ascontiguousarray(
            ch2_full[e].reshape(3, P, C)).astype(bf16)                 # (3,128,1000)
        oh_p = np.zeros((B, E), dtype=f32)
        oh_p[:, e] = 1.0
        m = {
            "xt": xt_p, "x2b": x2b, "xft": xft_p,
            "w1": w1[e].astype(bf16), "w2": w2[e].astype(bf16),
            "sw1": sw1_e, "sw2": sw2_p, "ch1": ch1_p, "ch2": ch2_p, "oh": oh_p,
        }
        if include_bias:
            m["b1d"] = np.asarray(inputs["b1"][e], f32).reshape(1, KD)
            m["b2d"] = np.asarray(inputs["b2"][e], f32).reshape(1, KD)
            m["swb1d"] = np.asarray(
                inputs["sw_b1"], f32).reshape(1, ND)[:, e * SWC:(e + 1) * SWC]
            m["swb2d"] = np.asarray(inputs["sw_b2"], f32).reshape(1, E)
            m["chb1d"] = np.asarray(
                inputs["ch_b1"], f32).reshape(1, KD)[:, e * CH1C:(e + 1) * CH1C]
        in_maps.append(m)
    return in_maps


def _need_bias(inputs) -> bool:
    return any(
        float(np.abs(np.asarray(inputs[k])).max()) != 0.0
        for k in ("b1", "b2", "sw_b1", "sw_b2", "ch_b1")
    )


def run(inputs: dict, **run_kwargs):
    """Run on the 8 cores; returns (full_output, BassKernelResults)."""
    include_bias = _need_bias(inputs)
    nc = _get_nc(include_bias)
    in_maps = _pack_inputs(inputs, include_bias)
    res = run_bass_kernel_spmd(nc, in_maps, core_ids=list(range(NCORES)), **run_kwargs)
    out = np.zeros((B, C), dtype=np.float64)
    for e in range(NCORES):
        out += res.results[e]["outp"].astype(np.float64)
    out += np.asarray(inputs["ch_b2"], np.float64)
    return out.astype(np.float32), res


def kernel(**inputs) -> np.ndarray:
    out, _ = run(inputs)
    return out
